# revision 5
# baseline (speedup 1.0000x reference)
"""Trainium2 Bass kernel for a gated linear recurrence (associative scan).

Problem: state_i = gates_i * state_{i-1} + inputs_i along the sequence axis,
elementwise in (batch, hidden). Shapes: gates/inputs [4, 4096, 4096] f32,
prev [4, 1, 4096] f32, out [4, 4096, 4096] f32.

Strategy:
  - Tensor-parallel: shard hidden dim D=4096 into 8 slices of 512, one per
    NeuronCore (the recurrence is elementwise in D -> zero communication).
  - Host-side, re-lay each core's slice as [B * (512/128), 128, S] so the
    sequence axis is contiguous in DRAM. Every device DMA is then a fully
    contiguous 2 MiB transfer ([128 partitions x 4096 f32]).
  - On-device, each [128, 4096] tile is one hardware TensorTensorScanArith
    instruction (op0=mult, op1=add) on the vector engine -- exactly this
    recurrence, with fp32 state feedback. initial = prev column.
  - The kernel is HBM-bound: 96 MiB of DMA per core vs ~50 us of DVE work.
"""

import os
import numpy as np

B, S, D = 4, 4096, 4096
N_CORES = 8
D_SH = D // N_CORES          # 512 hidden channels per core
PCH = D_SH // 128            # 4 partition-chunks per core
NT = B * PCH                 # 16 scan tiles of [128, S] per core

_state = {}


def _build_bass():
    import concourse.bacc as bacc
    import concourse.tile as tile
    from concourse import mybir

    f32 = mybir.dt.float32
    # Bacc (not raw Bass): its compile() legalizes multi-wait instructions
    # into EventSemaphore preludes -- the DVE ISA structs only carry one
    # sync-wait slot.
    nc = bacc.Bacc("TRN2", target_bir_lowering=False)

    g_d = nc.dram_tensor("gates_t", [NT * 128, S], f32, kind="ExternalInput")
    x_d = nc.dram_tensor("inputs_t", [NT * 128, S], f32, kind="ExternalInput")
    p_d = nc.dram_tensor("prev_t", [128, NT], f32, kind="ExternalInput")
    o_d = nc.dram_tensor("out_t", [NT * 128, S], f32, kind="ExternalOutput")
    g_ap, x_ap, p_ap, o_ap = g_d.ap(), x_d.ap(), p_d.ap(), o_d.ap()

    with tile.TileContext(nc) as tc:
        with (
            tc.tile_pool(name="io", bufs=3) as io_pool,
            tc.tile_pool(name="prev", bufs=1) as prev_pool,
        ):
            prev_sb = prev_pool.tile([128, NT], f32)
            nc.sync.dma_start(out=prev_sb[:], in_=p_ap[:, :])
            for i in range(NT):
                g_t = io_pool.tile([128, S], f32, tag="g")
                nc.sync.dma_start(out=g_t[:], in_=g_ap[i * 128 : (i + 1) * 128, :])
                x_t = io_pool.tile([128, S], f32, tag="x")
                nc.sync.dma_start(out=x_t[:], in_=x_ap[i * 128 : (i + 1) * 128, :])
                o_t = io_pool.tile([128, S], f32, tag="o")
                nc.vector.tensor_tensor_scan(
                    out=o_t[:],
                    data0=g_t[:],
                    data1=x_t[:],
                    initial=prev_sb[:, i : i + 1],
                    op0=mybir.AluOpType.mult,
                    op1=mybir.AluOpType.add,
                )
                nc.scalar.dma_start(out=o_ap[i * 128 : (i + 1) * 128, :], in_=o_t[:])
    nc.compile()
    return nc


def _shard_host(gates, inputs, prev):
    # [B, S, D] -> [B, D, S]: one big strided copy per tensor, then per-core
    # slices are cheap near-contiguous copies.
    gt = np.ascontiguousarray(gates.transpose(0, 2, 1))
    xt = np.ascontiguousarray(inputs.transpose(0, 2, 1))
    pv = prev[:, 0, :]  # [B, D]
    in_maps = []
    for c in range(N_CORES):
        sl = slice(c * D_SH, (c + 1) * D_SH)
        # row order (b, chunk, p): row i*128+p with i = b*PCH + chunk
        gc = np.ascontiguousarray(gt[:, sl, :]).reshape(NT * 128, S)
        xc = np.ascontiguousarray(xt[:, sl, :]).reshape(NT * 128, S)
        # prev_t[p, i] = prev[b, d0 + chunk*128 + p],  i = b*PCH + chunk
        pc = np.ascontiguousarray(
            pv[:, sl].reshape(B, PCH, 128).transpose(2, 0, 1).reshape(128, NT)
        )
        in_maps.append({"gates_t": gc, "inputs_t": xc, "prev_t": pc})
    return in_maps


def _gather_host(results):
    out_t = np.empty((B, D, S), np.float32)
    for c in range(N_CORES):
        sl = slice(c * D_SH, (c + 1) * D_SH)
        out_t[:, sl, :] = results[c]["out_t"].reshape(B, D_SH, S)
    return np.ascontiguousarray(out_t.transpose(0, 2, 1))


def _ntff_hook():
    """Slim NTFF profile hook over libaxon_pjrt.so (the image's antenv lacks
    axon_hooks, so run_bass_kernel_spmd's own trace path is unavailable)."""
    import ctypes
    import contextlib

    try:
        lib = ctypes.CDLL("/opt/axon/libaxon_pjrt.so")
        if not hasattr(lib, "axon_start_nrt_profile"):
            return None
    except OSError:
        return None
    lib.axon_start_nrt_profile.argtypes = [
        ctypes.POINTER(ctypes.c_int64),
        ctypes.c_size_t,
    ]
    lib.axon_start_nrt_profile.restype = ctypes.c_int64
    lib.axon_stop_nrt_profile.argtypes = [ctypes.c_char_p]
    lib.axon_stop_nrt_profile.restype = ctypes.c_int64

    @contextlib.contextmanager
    def _hook(output_dir, device_ids):
        import jax

        jax.devices()
        if device_ids:
            ids = (ctypes.c_int64 * len(device_ids))(*device_ids)
            rc = lib.axon_start_nrt_profile(ids, len(device_ids))
        else:
            rc = lib.axon_start_nrt_profile(None, 0)
        if rc != 0:
            raise RuntimeError(f"axon_start_nrt_profile rc={rc}")
        try:
            yield
        finally:
            n = lib.axon_stop_nrt_profile(str(output_dir).encode())
            print(f"profile: {n} file(s) written to {output_dir}")

    return _hook


def _extract_profile(nc, neff_dir, cores=(0,)):
    import gauge.profiler
    from concourse._compat import FishPath

    profile = gauge.profiler.Profile(
        profile_path=FishPath(neff_dir),
        kernel_dev_mode=True,
        profile_on_exit=False,
        bass_kernel=nc.m,
        offline_processing=True,
        fname="*_body*",
    )
    results = profile.to_perfetto(model_index=tuple(cores))
    info = {
        "exec_time_ns": max(r.exec_time_ns for r in results),
        "per_core_ns": {c: r.exec_time_ns for c, r in zip(cores, results)},
        "trace_paths": [r.trace_path for r in results],
        "scope_times": [r.scope_times for r in results],
    }
    return info


def run(gates, inputs, prev, trace=False, trace_cores=(0,)):
    """Returns (out [B,S,D] f32, profile-info dict or None)."""
    from concourse.bass_utils import run_bass_kernel_spmd

    if "nc" not in _state:
        _state["nc"] = _build_bass()
    nc = _state["nc"]
    in_maps = _shard_host(
        np.asarray(gates, np.float32),
        np.asarray(inputs, np.float32),
        np.asarray(prev, np.float32),
    )
    prof = None
    if trace:
        hook = _ntff_hook()
        if hook is not None:
            import tempfile

            from concourse import bass2jax

            neff_dir = tempfile.mkdtemp(prefix="scan_ntff_")
            with hook(neff_dir, list(trace_cores)):
                results = bass2jax.run_bass_via_pjrt(nc, in_maps, n_cores=N_CORES)
            try:
                prof = _extract_profile(nc, neff_dir, cores=trace_cores)
            except Exception as e:  # profiling must never break the run
                print(f"profile extraction failed: {e!r}")
            return _gather_host(results), prof
    res = run_bass_kernel_spmd(_state["nc"], in_maps, list(range(N_CORES)), trace=False)
    return _gather_host(res.results), prof


def kernel(gates, inputs, prev):
    trace = bool(int(os.environ.get("SCAN_TRACE", "0")))
    out, _ = run(gates, inputs, prev, trace=trace)
    return out


# revision 8
# speedup vs baseline: 1.1885x; 1.1885x over previous
"""Trainium2 Bass kernel for a gated linear recurrence (associative scan).

Problem: state_i = gates_i * state_{i-1} + inputs_i along the sequence axis,
elementwise in (batch, hidden). Shapes: gates/inputs [4, 4096, 4096] f32,
prev [4, 1, 4096] f32, out [4, 4096, 4096] f32.

Strategy:
  - Tensor-parallel: shard hidden dim D=4096 into 8 slices of 512, one per
    NeuronCore (the recurrence is elementwise in D -> zero communication).
  - Host-side, re-lay each core's slice as [B * (512/128), 128, S] so the
    sequence axis is contiguous in DRAM. Every device DMA is then a fully
    contiguous 2 MiB transfer ([128 partitions x 4096 f32]).
  - On-device, each [128, 4096] tile is one hardware TensorTensorScanArith
    instruction (op0=mult, op1=add) on the vector engine -- exactly this
    recurrence, with fp32 state feedback. initial = prev column.
  - The kernel is HBM-bound: 96 MiB of DMA per core vs ~50 us of DVE work.
"""

import os
import numpy as np

B, S, D = 4, 4096, 4096
N_CORES = 8
D_SH = D // N_CORES          # 512 hidden channels per core
PCH = D_SH // 128            # 4 partition-chunks per core
NT = B * PCH                 # 16 scan tiles of [128, S] per core

_state = {}


def _build_bass():
    import concourse.bacc as bacc
    import concourse.tile as tile
    from concourse import mybir

    f32 = mybir.dt.float32
    # Bacc (not raw Bass): its compile() legalizes multi-wait instructions
    # into EventSemaphore preludes -- the DVE ISA structs only carry one
    # sync-wait slot.
    nc = bacc.Bacc("TRN2", target_bir_lowering=False)

    g_d = nc.dram_tensor("gates_t", [NT * 128, S], f32, kind="ExternalInput")
    x_d = nc.dram_tensor("inputs_t", [NT * 128, S], f32, kind="ExternalInput")
    p_d = nc.dram_tensor("prev_t", [128, NT], f32, kind="ExternalInput")
    o_d = nc.dram_tensor("out_t", [NT * 128, S], f32, kind="ExternalOutput")
    g_ap, x_ap, p_ap, o_ap = g_d.ap(), x_d.ap(), p_d.ap(), o_d.ap()

    with tile.TileContext(nc) as tc:
        with (
            tc.tile_pool(name="io", bufs=3) as io_pool,
            tc.tile_pool(name="tail", bufs=1) as tail_pool,
            tc.tile_pool(name="prev", bufs=1) as prev_pool,
        ):
            prev_sb = prev_pool.tile([128, NT], f32)
            nc.sync.dma_start(out=prev_sb[:], in_=p_ap[:, :])
            for i in range(NT - 1):
                g_t = io_pool.tile([128, S], f32, tag="g")
                nc.sync.dma_start(out=g_t[:], in_=g_ap[i * 128 : (i + 1) * 128, :])
                x_t = io_pool.tile([128, S], f32, tag="x")
                nc.sync.dma_start(out=x_t[:], in_=x_ap[i * 128 : (i + 1) * 128, :])
                o_t = io_pool.tile([128, S], f32, tag="o")
                nc.vector.tensor_tensor_scan(
                    out=o_t[:],
                    data0=g_t[:],
                    data1=x_t[:],
                    initial=prev_sb[:, i : i + 1],
                    op0=mybir.AluOpType.mult,
                    op1=mybir.AluOpType.add,
                )
                nc.scalar.dma_start(out=o_ap[i * 128 : (i + 1) * 128, :], in_=o_t[:])
            # Last tile: chunk load/scan/store so the final store doesn't sit
            # behind one monolithic 8.8us scan after the last load lands.
            CH = 4
            CS = S // CH
            i = NT - 1
            rows = slice(i * 128, (i + 1) * 128)
            g_cs, x_cs, o_cs = [], [], []
            for c in range(CH):
                cols = slice(c * CS, (c + 1) * CS)
                g_c = tail_pool.tile([128, CS], f32, tag=f"gc{c}")
                nc.sync.dma_start(out=g_c[:], in_=g_ap[rows, cols])
                x_c = tail_pool.tile([128, CS], f32, tag=f"xc{c}")
                nc.sync.dma_start(out=x_c[:], in_=x_ap[rows, cols])
                g_cs.append(g_c)
                x_cs.append(x_c)
            for c in range(CH):
                cols = slice(c * CS, (c + 1) * CS)
                o_c = tail_pool.tile([128, CS], f32, tag=f"oc{c}")
                init = prev_sb[:, i : i + 1] if c == 0 else o_cs[-1][:, CS - 1 : CS]
                nc.vector.tensor_tensor_scan(
                    out=o_c[:],
                    data0=g_cs[c][:],
                    data1=x_cs[c][:],
                    initial=init,
                    op0=mybir.AluOpType.mult,
                    op1=mybir.AluOpType.add,
                )
                o_cs.append(o_c)
                nc.scalar.dma_start(out=o_ap[rows, cols], in_=o_c[:])
    nc.compile()
    return nc


def _shard_host(gates, inputs, prev):
    # [B, S, D] -> [B, D, S]: one big strided copy per tensor, then per-core
    # slices are cheap near-contiguous copies.
    gt = np.ascontiguousarray(gates.transpose(0, 2, 1))
    xt = np.ascontiguousarray(inputs.transpose(0, 2, 1))
    pv = prev[:, 0, :]  # [B, D]
    in_maps = []
    for c in range(N_CORES):
        sl = slice(c * D_SH, (c + 1) * D_SH)
        # row order (b, chunk, p): row i*128+p with i = b*PCH + chunk
        gc = np.ascontiguousarray(gt[:, sl, :]).reshape(NT * 128, S)
        xc = np.ascontiguousarray(xt[:, sl, :]).reshape(NT * 128, S)
        # prev_t[p, i] = prev[b, d0 + chunk*128 + p],  i = b*PCH + chunk
        pc = np.ascontiguousarray(
            pv[:, sl].reshape(B, PCH, 128).transpose(2, 0, 1).reshape(128, NT)
        )
        in_maps.append({"gates_t": gc, "inputs_t": xc, "prev_t": pc})
    return in_maps


def _gather_host(results):
    out_t = np.empty((B, D, S), np.float32)
    for c in range(N_CORES):
        sl = slice(c * D_SH, (c + 1) * D_SH)
        out_t[:, sl, :] = results[c]["out_t"].reshape(B, D_SH, S)
    return np.ascontiguousarray(out_t.transpose(0, 2, 1))


def _ntff_hook():
    """Slim NTFF profile hook over libaxon_pjrt.so (the image's antenv lacks
    axon_hooks, so run_bass_kernel_spmd's own trace path is unavailable)."""
    import ctypes
    import contextlib

    try:
        lib = ctypes.CDLL("/opt/axon/libaxon_pjrt.so")
        if not hasattr(lib, "axon_start_nrt_profile"):
            return None
    except OSError:
        return None
    lib.axon_start_nrt_profile.argtypes = [
        ctypes.POINTER(ctypes.c_int64),
        ctypes.c_size_t,
    ]
    lib.axon_start_nrt_profile.restype = ctypes.c_int64
    lib.axon_stop_nrt_profile.argtypes = [ctypes.c_char_p]
    lib.axon_stop_nrt_profile.restype = ctypes.c_int64

    @contextlib.contextmanager
    def _hook(output_dir, device_ids):
        import jax

        jax.devices()
        if device_ids:
            ids = (ctypes.c_int64 * len(device_ids))(*device_ids)
            rc = lib.axon_start_nrt_profile(ids, len(device_ids))
        else:
            rc = lib.axon_start_nrt_profile(None, 0)
        if rc != 0:
            raise RuntimeError(f"axon_start_nrt_profile rc={rc}")
        try:
            yield
        finally:
            n = lib.axon_stop_nrt_profile(str(output_dir).encode())
            print(f"profile: {n} file(s) written to {output_dir}")

    return _hook


def _extract_profile(nc, neff_dir, cores=(0,)):
    import gauge.profiler
    from concourse._compat import FishPath

    profile = gauge.profiler.Profile(
        profile_path=FishPath(neff_dir),
        kernel_dev_mode=True,
        profile_on_exit=False,
        bass_kernel=nc.m,
        offline_processing=True,
        fname="*_body*",
    )
    results = profile.to_perfetto(model_index=tuple(cores))
    info = {
        "exec_time_ns": max(r.exec_time_ns for r in results),
        "per_core_ns": {c: r.exec_time_ns for c, r in zip(cores, results)},
        "trace_paths": [r.trace_path for r in results],
        "scope_times": [r.scope_times for r in results],
    }
    return info


def run(gates, inputs, prev, trace=False, trace_cores=(0,)):
    """Returns (out [B,S,D] f32, profile-info dict or None)."""
    from concourse.bass_utils import run_bass_kernel_spmd

    if "nc" not in _state:
        _state["nc"] = _build_bass()
    nc = _state["nc"]
    in_maps = _shard_host(
        np.asarray(gates, np.float32),
        np.asarray(inputs, np.float32),
        np.asarray(prev, np.float32),
    )
    prof = None
    if trace:
        hook = _ntff_hook()
        if hook is not None:
            import tempfile

            from concourse import bass2jax

            neff_dir = tempfile.mkdtemp(prefix="scan_ntff_")
            with hook(neff_dir, list(trace_cores)):
                results = bass2jax.run_bass_via_pjrt(nc, in_maps, n_cores=N_CORES)
            try:
                prof = _extract_profile(nc, neff_dir, cores=trace_cores)
            except Exception as e:  # profiling must never break the run
                print(f"profile extraction failed: {e!r}")
            return _gather_host(results), prof
    res = run_bass_kernel_spmd(_state["nc"], in_maps, list(range(N_CORES)), trace=False)
    return _gather_host(res.results), prof


def kernel(gates, inputs, prev):
    trace = bool(int(os.environ.get("SCAN_TRACE", "0")))
    out, _ = run(gates, inputs, prev, trace=trace)
    return out
